# revision 32
# baseline (speedup 1.0000x reference)
"""LATTE GNN message-passing layer on 8 Trainium2 NeuronCores.

Algorithm (per relation m, with per-segment-constant terms cancelled from the
softmax):
    l = x@Wl + bl ; r = x@Wr + br
    ss_m[n,h]   = sum_c lrelu(l)[n,h*32+c] * attn[m,h,C+c] * sharpen[m]
    u_m[n,h]    = exp(ss_m[n,h])                      (dst-score cancels in softmax)
    z_m[n,hc]   = u_m[n,h] * l[n,hc]
    denom[n,h]  = sum_{e:dst=n} u_m[src_e,h]
    num[n,hc]   = sum_{e:dst=n} z_m[src_e,hc]
    emb_m       = num / (denom + eps)
    out = relu(emb0*beta0 + emb1*beta1 + r*beta2),  beta = softmax(x@(Wr@Wbeta.T)+brb)

Sharding: nodes split into 8 shards of 6272 (x padded to 50176 rows); each core
receives only its own x shard (bf16 over the wire), computes the dense
per-node tables (z|u packed as 132-f32 rows, one table per relation) for its
shard, then two on-device AllGathers assemble the full node tables on every
core.  Edges are partitioned by destination shard (local dst windows of 128);
each core gathers rows by global src id via indirect DMA and scatter-adds into
per-destination-window PSUM accumulators using one-hot matmuls.  Edge tables
are packed one int32 per edge slot: src | dst_lane<<16 (dst_lane 255 =
padding).  Output is written bf16 and widened on the host.

Execution: the Bass graph is compiled once through the same bass2jax/PJRT
path that bass_utils.run_bass_kernel_spmd uses under axon (shard_map over the
8 cores with donated output buffers); the jitted executable is cached at
module level and the previous call's output buffers are re-donated so warm
calls move only the real inputs/outputs over the host link.
"""

import numpy as np

N = 50000
D = 128
H = 4
C = 32
NCORES = 8
SH = 6272            # nodes per shard = 49 * 128
NPAD = SH * NCORES   # 50176
W = 49               # 128-node dst windows per shard
GW = W * NCORES      # 392 global windows
TW = 18              # gather/matmul tiles of 128 edges per window (padded)
EPS = 1e-12
ZC = 132             # z-row: 128 z values + 4 u values
PAD_LANE = 255       # dst lane marking a padded edge slot


def _prep_edges_all(edge_index):
    """All-core edge tables for one relation, window-major over global
    destination windows: src ids [GW, 128, TW] uint16 (node ids < 65536) and
    dst lanes [GW, 128, TW] uint8 (255 = padded slot)."""
    src = np.asarray(edge_index[0]).astype(np.int32)
    dst = np.asarray(edge_index[1]).astype(np.int32)
    # sort by full dst: within a window, slots are lane-ascending, so the
    # lane of slot j is recoverable from per-lane [start, end) count ranges.
    # dst < 65536, and numpy's stable sort on uint16 keys is radix (~2x
    # faster than introsort on int32)
    order = np.argsort(dst.astype(np.uint16), kind="stable")
    s = src[order]
    ds = dst[order]
    gws = ds >> 7
    cnt = np.bincount(gws, minlength=GW)
    assert cnt.max() <= TW * 128, f"window overflow: {cnt.max()} > {TW * 128}"
    starts = np.zeros(GW, np.int32)
    starts[1:] = np.cumsum(cnt[:-1], dtype=np.int32)
    offs = np.arange(GW, dtype=np.int32) * (128 * TW) - starts
    flat = np.arange(len(s), dtype=np.int32) + offs[gws]
    su = np.zeros(GW * 128 * TW, dtype=np.uint16)
    su[flat] = s.astype(np.uint16)
    nodecnt = np.bincount(dst, minlength=NPAD).reshape(GW, 128)
    ends = np.cumsum(nodecnt, axis=1, dtype=np.int32)
    cts = np.empty((GW, 1, 256), dtype=np.float32)
    cts[:, 0, :128] = ends - nodecnt
    cts[:, 0, 128:] = ends
    return su.reshape(GW, 128, TW), cts


def _build_graph():
    import concourse.bass as bass
    import concourse.mybir as mybir
    from concourse.bacc import Bacc
    from concourse.tile import TileContext
    from concourse.masks import make_identity

    f32 = mybir.dt.float32
    bf16 = mybir.dt.bfloat16
    i32 = mybir.dt.int32
    AF = mybir.ActivationFunctionType
    OP = mybir.AluOpType

    nc = Bacc(num_devices=NCORES)
    P_x = nc.declare_dram_parameter("x", [SH, D], bf16, isOutput=False)
    P_Wl = nc.declare_dram_parameter("Wl", [D, D], bf16, isOutput=False)
    P_Wr = nc.declare_dram_parameter("Wr", [D, D], bf16, isOutput=False)
    P_Wrb = nc.declare_dram_parameter("Wrb", [D, 3], f32, isOutput=False)
    P_A = nc.declare_dram_parameter("A", [D, 8], f32, isOutput=False)
    P_blr = nc.declare_dram_parameter("blr", [1, D], f32, isOutput=False)
    P_brr = nc.declare_dram_parameter("brr", [1, D], f32, isOutput=False)
    P_brbr = nc.declare_dram_parameter("brbr", [1, 3], f32, isOutput=False)
    P_es = [nc.declare_dram_parameter(f"es{m}", [W, 128, TW], mybir.dt.uint16,
                                      isOutput=False) for m in (0, 1)]
    P_ct = [nc.declare_dram_parameter(f"ct{m}", [W, 1, 256], f32,
                                      isOutput=False) for m in (0, 1)]
    P_out = nc.declare_dram_parameter("out", [SH, D], bf16, isOutput=True)

    zloc = [nc.dram_tensor(f"zloc{m}", [SH, ZC], f32) for m in (0, 1)]
    zt = [nc.dram_tensor(f"zt{m}", [NPAD, ZC], f32, addr_space="Shared")
          for m in (0, 1)]

    with TileContext(nc) as tc:
        with tc.tile_pool(name="pers", bufs=1) as pers:
            ident = pers.tile([128, 128], f32, tag="ident")
            make_identity(nc, ident[:])
            iota_j = pers.tile([128, TW], i32, tag="iota_j")
            nc.gpsimd.iota(iota_j[:], pattern=[[1, TW]], base=0,
                           channel_multiplier=TW)
            iota_jf = pers.tile([128, TW], f32, tag="iota_jf")
            nc.vector.tensor_copy(iota_jf[:], iota_j[:])
            ones1 = pers.tile([1, 128], f32, tag="ones1")
            nc.vector.memset(ones1[:], 1.0)

            wl_b = pers.tile([128, 128], bf16, tag="wlb")
            nc.sync.dma_start(out=wl_b[:], in_=P_Wl[:, :])
            wl_t = pers.tile([128, 128], f32, tag="wl")
            nc.vector.tensor_copy(wl_t[:], wl_b[:])
            wr_b = pers.tile([128, 128], bf16, tag="wrb16")
            nc.sync.dma_start(out=wr_b[:], in_=P_Wr[:, :])
            wr_t = pers.tile([128, 128], f32, tag="wr")
            nc.vector.tensor_copy(wr_t[:], wr_b[:])
            wrb_t = pers.tile([128, 3], f32, tag="wrb")
            nc.sync.dma_start(out=wrb_t[:], in_=P_Wrb[:, :])
            A_t = pers.tile([128, 8], f32, tag="A")
            nc.sync.dma_start(out=A_t[:], in_=P_A[:, :])
            blr_t = pers.tile([1, 128], f32, tag="blr")
            nc.sync.dma_start(out=blr_t[:], in_=P_blr[:, :])
            brr_t = pers.tile([1, 128], f32, tag="brr")
            nc.sync.dma_start(out=brr_t[:], in_=P_brr[:, :])
            brbr_t = pers.tile([1, 3], f32, tag="brbr")
            nc.sync.dma_start(out=brbr_t[:], in_=P_brbr[:, :])

            r_own = pers.tile([128, W * 128], f32, tag="r_own")
            beta_sb = pers.tile([128, W * 3], f32, tag="beta_sb")
            acc = pers.tile([128, W * 128], f32, tag="acc")

            # ---------------- dense phase (own shard only) ----------------
            with tc.tile_pool(name="dsb", bufs=3) as dsb, \
                 tc.tile_pool(name="dpsA", bufs=2, space="PSUM") as dpsA, \
                 tc.tile_pool(name="dpsB", bufs=1, space="PSUM") as dpsB:
                for g in range(W):
                    sl = slice(g * 128, (g + 1) * 128)
                    xb = dsb.tile([128, 128], bf16, tag="xb")
                    nc.sync.dma_start(out=xb[:], in_=P_x[sl, :])
                    xt = dsb.tile([128, 128], f32, tag="xt")
                    nc.vector.tensor_copy(xt[:], xb[:])
                    xT_ps = dpsB.tile([128, 128], f32, tag="xTp")
                    nc.tensor.transpose(xT_ps[:], xt[:], ident[:])
                    xT = dsb.tile([128, 128], f32, tag="xT")
                    nc.scalar.copy(out=xT[:], in_=xT_ps[:])

                    l_ps = dpsA.tile([128, 128], f32, tag="lp")
                    nc.tensor.matmul(out=l_ps[:], lhsT=xT[:], rhs=wl_t[:],
                                     start=True, stop=False)
                    nc.tensor.matmul(out=l_ps[:], lhsT=ones1[:], rhs=blr_t[:],
                                     start=False, stop=True)

                    lr = dsb.tile([128, 128], f32, tag="lr")
                    nc.vector.tensor_scalar_mul(lr[:], l_ps[:], 0.2)
                    nc.vector.tensor_tensor(out=lr[:], in0=lr[:], in1=l_ps[:],
                                            op=OP.max)
                    lrT_ps = dpsB.tile([128, 128], f32, tag="lrTp")
                    nc.tensor.transpose(lrT_ps[:], lr[:], ident[:])
                    lrT = dsb.tile([128, 128], f32, tag="lrT")
                    nc.scalar.copy(out=lrT[:], in_=lrT_ps[:])
                    ss_ps = dpsB.tile([128, 8], f32, tag="ssp")
                    nc.tensor.matmul(out=ss_ps[:], lhsT=lrT[:], rhs=A_t[:],
                                     start=True, stop=True)
                    u = dsb.tile([128, 8], f32, tag="u")
                    nc.scalar.activation(u[:], ss_ps[:], AF.Exp)

                    zu = dsb.tile([128, 2 * ZC], f32, tag="zu")
                    for m in (0, 1):
                        nc.vector.tensor_tensor(
                            out=zu[:, m * ZC:m * ZC + 128].rearrange(
                                "p (h c) -> p h c", h=4),
                            in0=l_ps[:, :].rearrange("p (h c) -> p h c", h=4),
                            in1=u[:, m * 4:(m + 1) * 4].to_broadcast([128, 4, 32]),
                            op=OP.mult)
                        nc.vector.tensor_copy(zu[:, m * ZC + 128:(m + 1) * ZC],
                                              u[:, m * 4:(m + 1) * 4])
                        nc.sync.dma_start(out=zloc[m][sl, :],
                                          in_=zu[:, m * ZC:(m + 1) * ZC])

                    r_ps = dpsB.tile([128, 128], f32, tag="rp")
                    nc.tensor.matmul(out=r_ps[:], lhsT=xT[:], rhs=wr_t[:],
                                     start=True, stop=False)
                    nc.tensor.matmul(out=r_ps[:], lhsT=ones1[:], rhs=brr_t[:],
                                     start=False, stop=True)
                    nc.scalar.copy(out=r_own[:, sl], in_=r_ps[:])

                    bl_ps = dpsB.tile([128, 3], f32, tag="blp")
                    nc.tensor.matmul(out=bl_ps[:], lhsT=xT[:], rhs=wrb_t[:],
                                     start=True, stop=False)
                    nc.tensor.matmul(out=bl_ps[:], lhsT=ones1[:], rhs=brbr_t[:],
                                     start=False, stop=True)
                    be = dsb.tile([128, 3], f32, tag="be")
                    nc.scalar.activation(be[:], bl_ps[:], AF.Exp)
                    bs = dsb.tile([128, 1], f32, tag="bs")
                    nc.vector.tensor_reduce(out=bs[:], in_=be[:],
                                            axis=mybir.AxisListType.X, op=OP.add)
                    brc = dsb.tile([128, 1], f32, tag="brc")
                    nc.vector.reciprocal(brc[:], bs[:])
                    nc.vector.tensor_tensor(
                        out=beta_sb[:, g * 3:(g + 1) * 3], in0=be[:],
                        in1=brc[:].to_broadcast([128, 3]), op=OP.mult)

            # assemble the full node tables on every core
            for m in (0, 1):
                nc.gpsimd.collective_compute(
                    "AllGather", mybir.AluOpType.bypass,
                    replica_groups=[list(range(NCORES))],
                    ins=[zloc[m][:, :].opt()],
                    outs=[zt[m][:, :].opt()])

            # phase barrier: collapse the dense-phase fan-in (all DMA lanes +
            # engines) into one sync point so edge-phase instructions stay
            # under the ISA per-instruction sync-wait limit
            with tc.tile_critical():
                nc.vector.memset(ones1[:], 1.0)

            # ---------------- edge phase ----------------
            with tc.tile_pool(name="esb", bufs=3) as esb, \
                 tc.tile_pool(name="eps", bufs=2, space="PSUM") as eps, \
                 tc.tile_pool(name="epsB", bufs=2, space="PSUM") as epsB:
                for m in (0, 1):
                    for w in range(W):
                        ws = slice(w * 128, (w + 1) * 128)
                        pes = esb.tile([128, TW], mybir.dt.uint16, tag="pes")
                        nc.sync.dma_start(out=pes[:], in_=P_es[m][w])
                        ct = esb.tile([1, 256], f32, tag="ct")
                        nc.sync.dma_start(out=ct[:], in_=P_ct[m][w])
                        idx = esb.tile([128, TW], i32, tag="idx")
                        nc.vector.tensor_copy(idx[:], pes[:])
                        # broadcast the window's per-lane [start, end) rows
                        # across partitions via ones-outer-product matmuls
                        st_ps = epsB.tile([128, 128], f32, tag="stp")
                        nc.tensor.matmul(out=st_ps[:], lhsT=ones1[:],
                                         rhs=ct[0:1, 0:128], start=True,
                                         stop=True)
                        st_sb = esb.tile([128, 128], f32, tag="stsb")
                        nc.scalar.copy(out=st_sb[:], in_=st_ps[:])
                        en_ps = epsB.tile([128, 128], f32, tag="enp")
                        nc.tensor.matmul(out=en_ps[:], lhsT=ones1[:],
                                         rhs=ct[0:1, 128:256], start=True,
                                         stop=True)
                        en_sb = esb.tile([128, 128], f32, tag="ensb")
                        nc.scalar.copy(out=en_sb[:], in_=en_ps[:])
                        # M[p,t,n] = (start[n] <= j(p,t) < end[n]), j = p*TW+t
                        M = esb.tile([128, TW * 128], f32, tag="M")
                        nc.vector.tensor_tensor(
                            out=M[:].rearrange("p (t n) -> p t n", t=TW),
                            in0=iota_jf[:].to_broadcast([128, TW, 128]),
                            in1=st_sb[:, None, :].to_broadcast([128, TW, 128]),
                            op=OP.is_ge)
                        M2 = esb.tile([128, TW * 128], f32, tag="M2")
                        nc.vector.tensor_tensor(
                            out=M2[:].rearrange("p (t n) -> p t n", t=TW),
                            in0=iota_jf[:].to_broadcast([128, TW, 128]),
                            in1=en_sb[:, None, :].to_broadcast([128, TW, 128]),
                            op=OP.is_ge)
                        nc.vector.tensor_tensor(out=M[:], in0=M[:], in1=M2[:],
                                                op=OP.subtract)
                        gt = esb.tile([128, TW * ZC], f32, tag="gt")
                        for t in range(TW):
                            nc.gpsimd.indirect_dma_start(
                                out=gt[:, t * ZC:(t + 1) * ZC], out_offset=None,
                                in_=zt[m][:, :],
                                in_offset=bass.IndirectOffsetOnAxis(
                                    ap=idx[:, t:t + 1], axis=0))
                        ps = eps.tile([128, ZC], f32, tag="pw")
                        for t in range(TW):
                            nc.tensor.matmul(out=ps[:],
                                             lhsT=M[:, t * 128:(t + 1) * 128],
                                             rhs=gt[:, t * ZC:(t + 1) * ZC],
                                             start=(t == 0), stop=(t == TW - 1))
                        den = esb.tile([128, 4], f32, tag="den")
                        nc.vector.tensor_scalar_add(den[:], ps[:, 128:132], EPS)
                        rec = esb.tile([128, 4], f32, tag="rec")
                        nc.vector.reciprocal(rec[:], den[:])
                        ab = esb.tile([128, 4], f32, tag="ab")
                        nc.vector.tensor_tensor(
                            out=ab[:], in0=rec[:],
                            in1=beta_sb[:, w * 3 + m:w * 3 + m + 1].to_broadcast([128, 4]),
                            op=OP.mult)
                        if m == 0:
                            nc.vector.tensor_tensor(
                                out=acc[:, ws].rearrange("p (h c) -> p h c", h=4),
                                in0=ps[:, 0:128].rearrange("p (h c) -> p h c", h=4),
                                in1=ab[:].to_broadcast([128, 4, 32]), op=OP.mult)
                        else:
                            tmp = esb.tile([128, 128], f32, tag="tmp")
                            nc.vector.tensor_tensor(
                                out=tmp[:].rearrange("p (h c) -> p h c", h=4),
                                in0=ps[:, 0:128].rearrange("p (h c) -> p h c", h=4),
                                in1=ab[:].to_broadcast([128, 4, 32]), op=OP.mult)
                            nc.vector.tensor_tensor(out=acc[:, ws], in0=acc[:, ws],
                                                    in1=tmp[:], op=OP.add)

                for w in range(W):
                    ws = slice(w * 128, (w + 1) * 128)
                    tmp = esb.tile([128, 128], f32, tag="tmp")
                    nc.vector.tensor_tensor(
                        out=tmp[:], in0=r_own[:, ws],
                        in1=beta_sb[:, w * 3 + 2:w * 3 + 3].to_broadcast([128, 128]),
                        op=OP.mult)
                    nc.vector.tensor_tensor(out=tmp[:], in0=tmp[:], in1=acc[:, ws],
                                            op=OP.add)
                    ot = esb.tile([128, 128], bf16, tag="ot")
                    nc.scalar.activation(ot[:], tmp[:], AF.Relu)
                    nc.sync.dma_start(out=P_out[ws, :], in_=ot[:])

    nc.finalize()
    return nc


def _prep_x(inputs):
    """bf16 [NPAD, D]; axis 0 is the per-core concatenation that
    shard_map(in_specs=P("core")) slices back apart."""
    from concourse import mybir
    bf16 = mybir.dt.np(mybir.dt.bfloat16)
    x = np.asarray(inputs["x"], dtype=np.float32)
    x_pad = np.zeros((NPAD, D), dtype=bf16)
    x_pad[:N] = x.astype(bf16)
    return x_pad


def _prep_weights(inputs):
    Wl = np.ascontiguousarray(np.asarray(inputs["Wl"], dtype=np.float32))
    bl = np.asarray(inputs["bl"], dtype=np.float32)
    Wr = np.ascontiguousarray(np.asarray(inputs["Wr"], dtype=np.float32))
    br = np.asarray(inputs["br"], dtype=np.float32)
    Wbeta = np.asarray(inputs["Wbeta"], dtype=np.float32)
    bbeta = np.asarray(inputs["bbeta"], dtype=np.float32)
    attn = np.asarray(inputs["attn"], dtype=np.float32)
    sharpen = np.asarray(inputs["sharpen"], dtype=np.float32)

    Wrb = np.ascontiguousarray(Wr @ Wbeta.T)             # [128, 3]
    brb = (br @ Wbeta.T + bbeta).astype(np.float32)      # [3]
    A = np.zeros((D, 8), dtype=np.float32)
    for m in (0, 1):
        aj = attn[m][:, C:]                              # [H, C]
        for h in range(H):
            A[h * C:(h + 1) * C, m * 4 + h] = aj[h] * sharpen[m]

    from concourse import mybir
    bf16 = mybir.dt.np(mybir.dt.bfloat16)

    def rep(a):
        return np.concatenate([a] * NCORES, axis=0)

    return {
        "Wl": rep(Wl.astype(bf16)), "Wr": rep(Wr.astype(bf16)),
        "Wrb": rep(Wrb), "A": rep(A),
        "blr": rep(bl[None, :]), "brr": rep(br[None, :]),
        "brbr": rep(brb[None, :]),
    }


_EXEC = None


def _get_exec():
    """Build (once) the jitted 8-core executable, mirroring the axon branch
    of bass_utils.run_bass_kernel_spmd / bass2jax.run_bass_via_pjrt."""
    global _EXEC
    if _EXEC is not None:
        return _EXEC
    import jax
    from jax.sharding import Mesh, PartitionSpec
    from jax.experimental.shard_map import shard_map
    from concourse import mybir
    from concourse.bass2jax import (_bass_exec_p, install_neuronx_cc_hook,
                                    partition_id_tensor)

    nc = _build_graph()
    install_neuronx_cc_hook()
    partition_name = (nc.partition_id_tensor.name
                      if nc.partition_id_tensor else None)
    in_names, out_names, out_avals, zero_outs = [], [], [], []
    for alloc in nc.m.functions[0].allocations:
        if not isinstance(alloc, mybir.MemoryLocationSet):
            continue
        name = alloc.memorylocations[0].name
        if alloc.kind == "ExternalInput":
            if name != partition_name:
                in_names.append(name)
        elif alloc.kind == "ExternalOutput":
            out_names.append(name)
            shape = tuple(alloc.tensor_shape)
            dtype = mybir.dt.np(alloc.dtype)
            out_avals.append(jax.core.ShapedArray(shape, dtype))
            zero_outs.append(np.zeros((NCORES * shape[0], *shape[1:]), dtype))
    n_params = len(in_names)
    in_names_all = in_names + out_names + (
        [partition_name] if partition_name else [])
    donate = tuple(range(n_params, n_params + len(out_names)))

    def _body(*args):
        operands = list(args)
        if partition_name is not None:
            operands.append(partition_id_tensor())
        return tuple(_bass_exec_p.bind(
            *operands, out_avals=tuple(out_avals),
            in_names=tuple(in_names_all), out_names=tuple(out_names),
            lowering_input_output_aliases=(), sim_require_finite=True,
            sim_require_nnan=True, nc=nc))

    devices = jax.devices()[:NCORES]
    assert len(devices) == NCORES
    mesh = Mesh(np.asarray(devices), ("core",))
    nspec = n_params + len(out_names)
    jf = jax.jit(
        shard_map(_body, mesh=mesh, in_specs=(PartitionSpec("core"),) * nspec,
                  out_specs=(PartitionSpec("core"),) * len(out_names),
                  check_rep=False),
        donate_argnums=donate, keep_unused=True)
    from jax.sharding import NamedSharding
    _EXEC = {"jf": jf, "in_names": in_names, "out_names": out_names,
             "zero_outs": zero_outs, "donated": None,
             "sharding": NamedSharding(mesh, PartitionSpec("core"))}
    return _EXEC


class _Res:
    exec_time_ns = None


def run(inputs, trace=False):
    import jax
    ex = _get_exec()
    sh = ex["sharding"]
    # upload x (the largest input) asynchronously; the transfer overlaps the
    # host-side edge-table construction below
    g = {"x": jax.device_put(_prep_x(inputs), sh)}
    for k, v in _prep_weights(inputs).items():
        g[k] = jax.device_put(v, sh)
    for m, key in ((0, "edge_index0"), (1, "edge_index1")):
        su, cts = _prep_edges_all(inputs[key])
        g[f"es{m}"] = jax.device_put(su, sh)
        g[f"ct{m}"] = jax.device_put(cts, sh)
    args = [g[name] for name in ex["in_names"]]
    douts = ex["donated"] if ex["donated"] is not None else ex["zero_outs"]
    outs = ex["jf"](*args, *douts)
    # every output element is written by the kernel, so the previous call's
    # output buffers are valid donation targets for the next call
    ex["donated"] = list(outs)
    oi = ex["out_names"].index("out")
    try:
        # queue the host copy behind the running execution
        outs[oi].copy_to_host_async()
    except (AttributeError, RuntimeError):
        pass
    out = np.asarray(outs[oi]).reshape(NPAD, D)[:N]
    return out.astype(np.float32), _Res()


def kernel(**inputs) -> np.ndarray:
    out, _ = run(inputs)
    return out


# revision 34
# speedup vs baseline: 1.0077x; 1.0077x over previous
"""LATTE GNN message-passing layer on 8 Trainium2 NeuronCores.

Algorithm (per relation m, with per-segment-constant terms cancelled from the
softmax):
    l = x@Wl + bl ; r = x@Wr + br
    ss_m[n,h]   = sum_c lrelu(l)[n,h*32+c] * attn[m,h,C+c] * sharpen[m]
    u_m[n,h]    = exp(ss_m[n,h])                      (dst-score cancels in softmax)
    z_m[n,hc]   = u_m[n,h] * l[n,hc]
    denom[n,h]  = sum_{e:dst=n} u_m[src_e,h]
    num[n,hc]   = sum_{e:dst=n} z_m[src_e,hc]
    emb_m       = num / (denom + eps)
    out = relu(emb0*beta0 + emb1*beta1 + r*beta2),  beta = softmax(x@(Wr@Wbeta.T)+brb)

Sharding: nodes split into 8 shards of 6272 (x padded to 50176 rows); each core
receives only its own x shard (bf16 over the wire), computes the dense
per-node tables (z|u packed as 132-f32 rows, one table per relation) for its
shard, then two on-device AllGathers assemble the full node tables on every
core.  Edges are partitioned by destination shard (local dst windows of 128);
each core gathers rows by global src id via indirect DMA and scatter-adds into
per-destination-window PSUM accumulators using one-hot matmuls.  Edge tables
are packed one int32 per edge slot: src | dst_lane<<16 (dst_lane 255 =
padding).  Output is written bf16 and widened on the host.

Execution: the Bass graph is compiled once through the same bass2jax/PJRT
path that bass_utils.run_bass_kernel_spmd uses under axon (shard_map over the
8 cores with donated output buffers); the jitted executable is cached at
module level and the previous call's output buffers are re-donated so warm
calls move only the real inputs/outputs over the host link.
"""

import numpy as np

N = 50000
D = 128
H = 4
C = 32
NCORES = 8
SH = 6272            # nodes per shard = 49 * 128
NPAD = SH * NCORES   # 50176
W = 49               # 128-node dst windows per shard
GW = W * NCORES      # 392 global windows
TW = 18              # gather/matmul tiles of 128 edges per window (padded)
EPS = 1e-12
ZC = 132             # z-row: 128 z values + 4 u values
PAD_LANE = 255       # dst lane marking a padded edge slot


def _prep_edges_all(edge_index):
    """All-core edge tables for one relation, window-major over global
    destination windows: src ids [GW, 128, TW] uint16 (node ids < 65536) and
    dst lanes [GW, 128, TW] uint8 (255 = padded slot)."""
    # node ids < 65536: uint16 halves the bytes moved by the sort and the
    # permutation gathers, and numpy's stable sort on uint16 keys is radix
    # (~2x faster than introsort on int32)
    src = np.asarray(edge_index[0]).astype(np.uint16)
    dst = np.asarray(edge_index[1]).astype(np.int32)
    dst16 = dst.astype(np.uint16)
    # sort by full dst: within a window, slots are lane-ascending, so the
    # lane of slot j is recoverable from per-lane [start, end) count ranges
    order = np.argsort(dst16, kind="stable")
    s = src[order]
    ds = dst16[order]
    gws = ds >> 7
    cnt = np.bincount(gws, minlength=GW)
    assert cnt.max() <= TW * 128, f"window overflow: {cnt.max()} > {TW * 128}"
    starts = np.zeros(GW, np.int32)
    starts[1:] = np.cumsum(cnt[:-1], dtype=np.int32)
    offs = np.arange(GW, dtype=np.int32) * (128 * TW) - starts
    flat = np.arange(len(s), dtype=np.int32) + offs[gws]
    su = np.zeros(GW * 128 * TW, dtype=np.uint16)
    su[flat] = s
    nodecnt = np.bincount(dst, minlength=NPAD).reshape(GW, 128)
    ends = np.cumsum(nodecnt, axis=1, dtype=np.int32)
    cts = np.empty((GW, 1, 256), dtype=np.float32)
    cts[:, 0, :128] = ends - nodecnt
    cts[:, 0, 128:] = ends
    return su.reshape(GW, 128, TW), cts


def _build_graph():
    import concourse.bass as bass
    import concourse.mybir as mybir
    from concourse.bacc import Bacc
    from concourse.tile import TileContext
    from concourse.masks import make_identity

    f32 = mybir.dt.float32
    bf16 = mybir.dt.bfloat16
    i32 = mybir.dt.int32
    AF = mybir.ActivationFunctionType
    OP = mybir.AluOpType

    nc = Bacc(num_devices=NCORES)
    P_x = nc.declare_dram_parameter("x", [SH, D], bf16, isOutput=False)
    P_Wl = nc.declare_dram_parameter("Wl", [D, D], bf16, isOutput=False)
    P_Wr = nc.declare_dram_parameter("Wr", [D, D], bf16, isOutput=False)
    P_Wrb = nc.declare_dram_parameter("Wrb", [D, 3], f32, isOutput=False)
    P_A = nc.declare_dram_parameter("A", [D, 8], f32, isOutput=False)
    P_blr = nc.declare_dram_parameter("blr", [1, D], f32, isOutput=False)
    P_brr = nc.declare_dram_parameter("brr", [1, D], f32, isOutput=False)
    P_brbr = nc.declare_dram_parameter("brbr", [1, 3], f32, isOutput=False)
    P_es = [nc.declare_dram_parameter(f"es{m}", [W, 128, TW], mybir.dt.uint16,
                                      isOutput=False) for m in (0, 1)]
    P_ct = [nc.declare_dram_parameter(f"ct{m}", [W, 1, 256], f32,
                                      isOutput=False) for m in (0, 1)]
    P_out = nc.declare_dram_parameter("out", [SH, D], bf16, isOutput=True)

    zloc = [nc.dram_tensor(f"zloc{m}", [SH, ZC], f32) for m in (0, 1)]
    zt = [nc.dram_tensor(f"zt{m}", [NPAD, ZC], f32, addr_space="Shared")
          for m in (0, 1)]

    with TileContext(nc) as tc:
        with tc.tile_pool(name="pers", bufs=1) as pers:
            ident = pers.tile([128, 128], f32, tag="ident")
            make_identity(nc, ident[:])
            iota_j = pers.tile([128, TW], i32, tag="iota_j")
            nc.gpsimd.iota(iota_j[:], pattern=[[1, TW]], base=0,
                           channel_multiplier=TW)
            iota_jf = pers.tile([128, TW], f32, tag="iota_jf")
            nc.vector.tensor_copy(iota_jf[:], iota_j[:])
            ones1 = pers.tile([1, 128], f32, tag="ones1")
            nc.vector.memset(ones1[:], 1.0)

            wl_b = pers.tile([128, 128], bf16, tag="wlb")
            nc.sync.dma_start(out=wl_b[:], in_=P_Wl[:, :])
            wl_t = pers.tile([128, 128], f32, tag="wl")
            nc.vector.tensor_copy(wl_t[:], wl_b[:])
            wr_b = pers.tile([128, 128], bf16, tag="wrb16")
            nc.sync.dma_start(out=wr_b[:], in_=P_Wr[:, :])
            wr_t = pers.tile([128, 128], f32, tag="wr")
            nc.vector.tensor_copy(wr_t[:], wr_b[:])
            wrb_t = pers.tile([128, 3], f32, tag="wrb")
            nc.sync.dma_start(out=wrb_t[:], in_=P_Wrb[:, :])
            A_t = pers.tile([128, 8], f32, tag="A")
            nc.sync.dma_start(out=A_t[:], in_=P_A[:, :])
            blr_t = pers.tile([1, 128], f32, tag="blr")
            nc.sync.dma_start(out=blr_t[:], in_=P_blr[:, :])
            brr_t = pers.tile([1, 128], f32, tag="brr")
            nc.sync.dma_start(out=brr_t[:], in_=P_brr[:, :])
            brbr_t = pers.tile([1, 3], f32, tag="brbr")
            nc.sync.dma_start(out=brbr_t[:], in_=P_brbr[:, :])

            r_own = pers.tile([128, W * 128], f32, tag="r_own")
            beta_sb = pers.tile([128, W * 3], f32, tag="beta_sb")
            acc = pers.tile([128, W * 128], f32, tag="acc")

            # ---------------- dense phase (own shard only) ----------------
            with tc.tile_pool(name="dsb", bufs=3) as dsb, \
                 tc.tile_pool(name="dpsA", bufs=2, space="PSUM") as dpsA, \
                 tc.tile_pool(name="dpsB", bufs=1, space="PSUM") as dpsB:
                for g in range(W):
                    sl = slice(g * 128, (g + 1) * 128)
                    xb = dsb.tile([128, 128], bf16, tag="xb")
                    nc.sync.dma_start(out=xb[:], in_=P_x[sl, :])
                    xt = dsb.tile([128, 128], f32, tag="xt")
                    nc.vector.tensor_copy(xt[:], xb[:])
                    xT_ps = dpsB.tile([128, 128], f32, tag="xTp")
                    nc.tensor.transpose(xT_ps[:], xt[:], ident[:])
                    xT = dsb.tile([128, 128], f32, tag="xT")
                    nc.scalar.copy(out=xT[:], in_=xT_ps[:])

                    l_ps = dpsA.tile([128, 128], f32, tag="lp")
                    nc.tensor.matmul(out=l_ps[:], lhsT=xT[:], rhs=wl_t[:],
                                     start=True, stop=False)
                    nc.tensor.matmul(out=l_ps[:], lhsT=ones1[:], rhs=blr_t[:],
                                     start=False, stop=True)

                    lr = dsb.tile([128, 128], f32, tag="lr")
                    nc.vector.tensor_scalar_mul(lr[:], l_ps[:], 0.2)
                    nc.vector.tensor_tensor(out=lr[:], in0=lr[:], in1=l_ps[:],
                                            op=OP.max)
                    lrT_ps = dpsB.tile([128, 128], f32, tag="lrTp")
                    nc.tensor.transpose(lrT_ps[:], lr[:], ident[:])
                    lrT = dsb.tile([128, 128], f32, tag="lrT")
                    nc.scalar.copy(out=lrT[:], in_=lrT_ps[:])
                    ss_ps = dpsB.tile([128, 8], f32, tag="ssp")
                    nc.tensor.matmul(out=ss_ps[:], lhsT=lrT[:], rhs=A_t[:],
                                     start=True, stop=True)
                    u = dsb.tile([128, 8], f32, tag="u")
                    nc.scalar.activation(u[:], ss_ps[:], AF.Exp)

                    zu = dsb.tile([128, 2 * ZC], f32, tag="zu")
                    for m in (0, 1):
                        nc.vector.tensor_tensor(
                            out=zu[:, m * ZC:m * ZC + 128].rearrange(
                                "p (h c) -> p h c", h=4),
                            in0=l_ps[:, :].rearrange("p (h c) -> p h c", h=4),
                            in1=u[:, m * 4:(m + 1) * 4].to_broadcast([128, 4, 32]),
                            op=OP.mult)
                        nc.vector.tensor_copy(zu[:, m * ZC + 128:(m + 1) * ZC],
                                              u[:, m * 4:(m + 1) * 4])
                        nc.sync.dma_start(out=zloc[m][sl, :],
                                          in_=zu[:, m * ZC:(m + 1) * ZC])

                    r_ps = dpsB.tile([128, 128], f32, tag="rp")
                    nc.tensor.matmul(out=r_ps[:], lhsT=xT[:], rhs=wr_t[:],
                                     start=True, stop=False)
                    nc.tensor.matmul(out=r_ps[:], lhsT=ones1[:], rhs=brr_t[:],
                                     start=False, stop=True)
                    nc.scalar.copy(out=r_own[:, sl], in_=r_ps[:])

                    bl_ps = dpsB.tile([128, 3], f32, tag="blp")
                    nc.tensor.matmul(out=bl_ps[:], lhsT=xT[:], rhs=wrb_t[:],
                                     start=True, stop=False)
                    nc.tensor.matmul(out=bl_ps[:], lhsT=ones1[:], rhs=brbr_t[:],
                                     start=False, stop=True)
                    be = dsb.tile([128, 3], f32, tag="be")
                    nc.scalar.activation(be[:], bl_ps[:], AF.Exp)
                    bs = dsb.tile([128, 1], f32, tag="bs")
                    nc.vector.tensor_reduce(out=bs[:], in_=be[:],
                                            axis=mybir.AxisListType.X, op=OP.add)
                    brc = dsb.tile([128, 1], f32, tag="brc")
                    nc.vector.reciprocal(brc[:], bs[:])
                    nc.vector.tensor_tensor(
                        out=beta_sb[:, g * 3:(g + 1) * 3], in0=be[:],
                        in1=brc[:].to_broadcast([128, 3]), op=OP.mult)

            # assemble the full node tables on every core
            for m in (0, 1):
                nc.gpsimd.collective_compute(
                    "AllGather", mybir.AluOpType.bypass,
                    replica_groups=[list(range(NCORES))],
                    ins=[zloc[m][:, :].opt()],
                    outs=[zt[m][:, :].opt()])

            # phase barrier: collapse the dense-phase fan-in (all DMA lanes +
            # engines) into one sync point so edge-phase instructions stay
            # under the ISA per-instruction sync-wait limit
            with tc.tile_critical():
                nc.vector.memset(ones1[:], 1.0)

            # ---------------- edge phase ----------------
            with tc.tile_pool(name="esb", bufs=3) as esb, \
                 tc.tile_pool(name="eps", bufs=2, space="PSUM") as eps, \
                 tc.tile_pool(name="epsB", bufs=2, space="PSUM") as epsB:
                for m in (0, 1):
                    for w in range(W):
                        ws = slice(w * 128, (w + 1) * 128)
                        pes = esb.tile([128, TW], mybir.dt.uint16, tag="pes")
                        nc.sync.dma_start(out=pes[:], in_=P_es[m][w])
                        ct = esb.tile([1, 256], f32, tag="ct")
                        nc.sync.dma_start(out=ct[:], in_=P_ct[m][w])
                        idx = esb.tile([128, TW], i32, tag="idx")
                        nc.vector.tensor_copy(idx[:], pes[:])
                        # broadcast the window's per-lane [start, end) rows
                        # across partitions via ones-outer-product matmuls
                        st_ps = epsB.tile([128, 128], f32, tag="stp")
                        nc.tensor.matmul(out=st_ps[:], lhsT=ones1[:],
                                         rhs=ct[0:1, 0:128], start=True,
                                         stop=True)
                        st_sb = esb.tile([128, 128], f32, tag="stsb")
                        nc.scalar.copy(out=st_sb[:], in_=st_ps[:])
                        en_ps = epsB.tile([128, 128], f32, tag="enp")
                        nc.tensor.matmul(out=en_ps[:], lhsT=ones1[:],
                                         rhs=ct[0:1, 128:256], start=True,
                                         stop=True)
                        en_sb = esb.tile([128, 128], f32, tag="ensb")
                        nc.scalar.copy(out=en_sb[:], in_=en_ps[:])
                        # M[p,t,n] = (start[n] <= j(p,t) < end[n]), j = p*TW+t
                        M = esb.tile([128, TW * 128], f32, tag="M")
                        nc.vector.tensor_tensor(
                            out=M[:].rearrange("p (t n) -> p t n", t=TW),
                            in0=iota_jf[:].to_broadcast([128, TW, 128]),
                            in1=st_sb[:, None, :].to_broadcast([128, TW, 128]),
                            op=OP.is_ge)
                        M2 = esb.tile([128, TW * 128], f32, tag="M2")
                        nc.vector.tensor_tensor(
                            out=M2[:].rearrange("p (t n) -> p t n", t=TW),
                            in0=iota_jf[:].to_broadcast([128, TW, 128]),
                            in1=en_sb[:, None, :].to_broadcast([128, TW, 128]),
                            op=OP.is_ge)
                        nc.vector.tensor_tensor(out=M[:], in0=M[:], in1=M2[:],
                                                op=OP.subtract)
                        gt = esb.tile([128, TW * ZC], f32, tag="gt")
                        for t in range(TW):
                            nc.gpsimd.indirect_dma_start(
                                out=gt[:, t * ZC:(t + 1) * ZC], out_offset=None,
                                in_=zt[m][:, :],
                                in_offset=bass.IndirectOffsetOnAxis(
                                    ap=idx[:, t:t + 1], axis=0))
                        ps = eps.tile([128, ZC], f32, tag="pw")
                        for t in range(TW):
                            nc.tensor.matmul(out=ps[:],
                                             lhsT=M[:, t * 128:(t + 1) * 128],
                                             rhs=gt[:, t * ZC:(t + 1) * ZC],
                                             start=(t == 0), stop=(t == TW - 1))
                        den = esb.tile([128, 4], f32, tag="den")
                        nc.vector.tensor_scalar_add(den[:], ps[:, 128:132], EPS)
                        rec = esb.tile([128, 4], f32, tag="rec")
                        nc.vector.reciprocal(rec[:], den[:])
                        ab = esb.tile([128, 4], f32, tag="ab")
                        nc.vector.tensor_tensor(
                            out=ab[:], in0=rec[:],
                            in1=beta_sb[:, w * 3 + m:w * 3 + m + 1].to_broadcast([128, 4]),
                            op=OP.mult)
                        if m == 0:
                            nc.vector.tensor_tensor(
                                out=acc[:, ws].rearrange("p (h c) -> p h c", h=4),
                                in0=ps[:, 0:128].rearrange("p (h c) -> p h c", h=4),
                                in1=ab[:].to_broadcast([128, 4, 32]), op=OP.mult)
                        else:
                            tmp = esb.tile([128, 128], f32, tag="tmp")
                            nc.vector.tensor_tensor(
                                out=tmp[:].rearrange("p (h c) -> p h c", h=4),
                                in0=ps[:, 0:128].rearrange("p (h c) -> p h c", h=4),
                                in1=ab[:].to_broadcast([128, 4, 32]), op=OP.mult)
                            nc.vector.tensor_tensor(out=acc[:, ws], in0=acc[:, ws],
                                                    in1=tmp[:], op=OP.add)

                for w in range(W):
                    ws = slice(w * 128, (w + 1) * 128)
                    tmp = esb.tile([128, 128], f32, tag="tmp")
                    nc.vector.tensor_tensor(
                        out=tmp[:], in0=r_own[:, ws],
                        in1=beta_sb[:, w * 3 + 2:w * 3 + 3].to_broadcast([128, 128]),
                        op=OP.mult)
                    nc.vector.tensor_tensor(out=tmp[:], in0=tmp[:], in1=acc[:, ws],
                                            op=OP.add)
                    ot = esb.tile([128, 128], bf16, tag="ot")
                    nc.scalar.activation(ot[:], tmp[:], AF.Relu)
                    nc.sync.dma_start(out=P_out[ws, :], in_=ot[:])

    nc.finalize()
    return nc


def _prep_x(inputs):
    """bf16 [NPAD, D]; axis 0 is the per-core concatenation that
    shard_map(in_specs=P("core")) slices back apart."""
    from concourse import mybir
    bf16 = mybir.dt.np(mybir.dt.bfloat16)
    x = np.asarray(inputs["x"], dtype=np.float32)
    x_pad = np.zeros((NPAD, D), dtype=bf16)
    x_pad[:N] = x.astype(bf16)
    return x_pad


def _prep_weights(inputs):
    Wl = np.ascontiguousarray(np.asarray(inputs["Wl"], dtype=np.float32))
    bl = np.asarray(inputs["bl"], dtype=np.float32)
    Wr = np.ascontiguousarray(np.asarray(inputs["Wr"], dtype=np.float32))
    br = np.asarray(inputs["br"], dtype=np.float32)
    Wbeta = np.asarray(inputs["Wbeta"], dtype=np.float32)
    bbeta = np.asarray(inputs["bbeta"], dtype=np.float32)
    attn = np.asarray(inputs["attn"], dtype=np.float32)
    sharpen = np.asarray(inputs["sharpen"], dtype=np.float32)

    Wrb = np.ascontiguousarray(Wr @ Wbeta.T)             # [128, 3]
    brb = (br @ Wbeta.T + bbeta).astype(np.float32)      # [3]
    A = np.zeros((D, 8), dtype=np.float32)
    for m in (0, 1):
        aj = attn[m][:, C:]                              # [H, C]
        for h in range(H):
            A[h * C:(h + 1) * C, m * 4 + h] = aj[h] * sharpen[m]

    from concourse import mybir
    bf16 = mybir.dt.np(mybir.dt.bfloat16)

    def rep(a):
        return np.concatenate([a] * NCORES, axis=0)

    return {
        "Wl": rep(Wl.astype(bf16)), "Wr": rep(Wr.astype(bf16)),
        "Wrb": rep(Wrb), "A": rep(A),
        "blr": rep(bl[None, :]), "brr": rep(br[None, :]),
        "brbr": rep(brb[None, :]),
    }


_EXEC = None


def _get_exec():
    """Build (once) the jitted 8-core executable, mirroring the axon branch
    of bass_utils.run_bass_kernel_spmd / bass2jax.run_bass_via_pjrt."""
    global _EXEC
    if _EXEC is not None:
        return _EXEC
    import jax
    from jax.sharding import Mesh, PartitionSpec
    from jax.experimental.shard_map import shard_map
    from concourse import mybir
    from concourse.bass2jax import (_bass_exec_p, install_neuronx_cc_hook,
                                    partition_id_tensor)

    nc = _build_graph()
    install_neuronx_cc_hook()
    partition_name = (nc.partition_id_tensor.name
                      if nc.partition_id_tensor else None)
    in_names, out_names, out_avals, zero_outs = [], [], [], []
    for alloc in nc.m.functions[0].allocations:
        if not isinstance(alloc, mybir.MemoryLocationSet):
            continue
        name = alloc.memorylocations[0].name
        if alloc.kind == "ExternalInput":
            if name != partition_name:
                in_names.append(name)
        elif alloc.kind == "ExternalOutput":
            out_names.append(name)
            shape = tuple(alloc.tensor_shape)
            dtype = mybir.dt.np(alloc.dtype)
            out_avals.append(jax.core.ShapedArray(shape, dtype))
            zero_outs.append(np.zeros((NCORES * shape[0], *shape[1:]), dtype))
    n_params = len(in_names)
    in_names_all = in_names + out_names + (
        [partition_name] if partition_name else [])
    donate = tuple(range(n_params, n_params + len(out_names)))

    def _body(*args):
        operands = list(args)
        if partition_name is not None:
            operands.append(partition_id_tensor())
        return tuple(_bass_exec_p.bind(
            *operands, out_avals=tuple(out_avals),
            in_names=tuple(in_names_all), out_names=tuple(out_names),
            lowering_input_output_aliases=(), sim_require_finite=True,
            sim_require_nnan=True, nc=nc))

    devices = jax.devices()[:NCORES]
    assert len(devices) == NCORES
    mesh = Mesh(np.asarray(devices), ("core",))
    nspec = n_params + len(out_names)
    jf = jax.jit(
        shard_map(_body, mesh=mesh, in_specs=(PartitionSpec("core"),) * nspec,
                  out_specs=(PartitionSpec("core"),) * len(out_names),
                  check_rep=False),
        donate_argnums=donate, keep_unused=True)
    from jax.sharding import NamedSharding
    _EXEC = {"jf": jf, "in_names": in_names, "out_names": out_names,
             "zero_outs": zero_outs, "donated": None,
             "sharding": NamedSharding(mesh, PartitionSpec("core"))}
    return _EXEC


class _Res:
    exec_time_ns = None


def run(inputs, trace=False):
    import jax
    ex = _get_exec()
    sh = ex["sharding"]
    # upload x (the largest input) asynchronously; the transfer overlaps the
    # host-side edge-table construction below
    g = {"x": jax.device_put(_prep_x(inputs), sh)}
    for k, v in _prep_weights(inputs).items():
        g[k] = jax.device_put(v, sh)
    for m, key in ((0, "edge_index0"), (1, "edge_index1")):
        su, cts = _prep_edges_all(inputs[key])
        g[f"es{m}"] = jax.device_put(su, sh)
        g[f"ct{m}"] = jax.device_put(cts, sh)
    args = [g[name] for name in ex["in_names"]]
    douts = ex["donated"] if ex["donated"] is not None else ex["zero_outs"]
    outs = ex["jf"](*args, *douts)
    # every output element is written by the kernel, so the previous call's
    # output buffers are valid donation targets for the next call
    ex["donated"] = list(outs)
    oi = ex["out_names"].index("out")
    try:
        # queue the host copy behind the running execution
        outs[oi].copy_to_host_async()
    except (AttributeError, RuntimeError):
        pass
    out = np.asarray(outs[oi]).reshape(NPAD, D)[:N]
    return out.astype(np.float32), _Res()


def kernel(**inputs) -> np.ndarray:
    out, _ = run(inputs)
    return out


# revision 35
# speedup vs baseline: 1.0456x; 1.0375x over previous
"""LATTE GNN message-passing layer on 8 Trainium2 NeuronCores.

Algorithm (per relation m, with per-segment-constant terms cancelled from the
softmax):
    l = x@Wl + bl ; r = x@Wr + br
    ss_m[n,h]   = sum_c lrelu(l)[n,h*32+c] * attn[m,h,C+c] * sharpen[m]
    u_m[n,h]    = exp(ss_m[n,h])                      (dst-score cancels in softmax)
    z_m[n,hc]   = u_m[n,h] * l[n,hc]
    denom[n,h]  = sum_{e:dst=n} u_m[src_e,h]
    num[n,hc]   = sum_{e:dst=n} z_m[src_e,hc]
    emb_m       = num / (denom + eps)
    out = relu(emb0*beta0 + emb1*beta1 + r*beta2),  beta = softmax(x@(Wr@Wbeta.T)+brb)

Sharding: nodes split into 8 shards of 6272 (x padded to 50176 rows); each core
receives only its own x shard (bf16 over the wire), computes the dense
per-node tables (z|u packed as 132-f32 rows, one table per relation) for its
shard, then two on-device AllGathers assemble the full node tables on every
core.  Edges are partitioned by destination shard (local dst windows of 128);
each core gathers rows by global src id via indirect DMA and scatter-adds into
per-destination-window PSUM accumulators using one-hot matmuls.  Edge tables
are packed one int32 per edge slot: src | dst_lane<<16 (dst_lane 255 =
padding).  Output is written bf16 and widened on the host.

Execution: the Bass graph is compiled once through the same bass2jax/PJRT
path that bass_utils.run_bass_kernel_spmd uses under axon (shard_map over the
8 cores with donated output buffers); the jitted executable is cached at
module level and the previous call's output buffers are re-donated so warm
calls move only the real inputs/outputs over the host link.
"""

import numpy as np

N = 50000
D = 128
H = 4
C = 32
NCORES = 8
SH = 6272            # nodes per shard = 49 * 128
NPAD = SH * NCORES   # 50176
W = 49               # 128-node dst windows per shard
GW = W * NCORES      # 392 global windows
TW = 18              # gather/matmul tiles of 128 edges per window (padded)
EPS = 1e-12
ZC = 132             # z-row: 128 z values + 4 u values
PAD_LANE = 255       # dst lane marking a padded edge slot


def _prep_edges_all(edge_index):
    """All-core edge tables for one relation, window-major over global
    destination windows: src ids [GW, 128, TW] uint16 (node ids < 65536) and
    dst lanes [GW, 128, TW] uint8 (255 = padded slot)."""
    # node ids < 65536: uint16 halves the bytes moved by the sort and the
    # permutation gathers, and numpy's stable sort on uint16 keys is radix
    # (~2x faster than introsort on int32)
    src = np.asarray(edge_index[0]).astype(np.uint16)
    dst = np.asarray(edge_index[1]).astype(np.int32)
    dst16 = dst.astype(np.uint16)
    # sort by full dst: within a window, slots are lane-ascending, so the
    # lane of slot j is recoverable from per-lane [start, end) count ranges
    order = np.argsort(dst16, kind="stable")
    s = src[order]
    ds = dst16[order]
    gws = ds >> 7
    cnt = np.bincount(gws, minlength=GW)
    assert cnt.max() <= TW * 128, f"window overflow: {cnt.max()} > {TW * 128}"
    starts = np.zeros(GW, np.int32)
    starts[1:] = np.cumsum(cnt[:-1], dtype=np.int32)
    offs = np.arange(GW, dtype=np.int32) * (128 * TW) - starts
    flat = np.arange(len(s), dtype=np.int32) + offs[gws]
    su = np.zeros(GW * 128 * TW, dtype=np.uint16)
    su[flat] = s
    nodecnt = np.bincount(dst, minlength=NPAD).reshape(GW, 128)
    ends = np.cumsum(nodecnt, axis=1, dtype=np.int32)
    cts = np.empty((GW, 1, 256), dtype=np.float32)
    cts[:, 0, :128] = ends - nodecnt
    cts[:, 0, 128:] = ends
    return su.reshape(GW, 128, TW), cts


def _build_graph():
    import concourse.bass as bass
    import concourse.mybir as mybir
    from concourse.bacc import Bacc
    from concourse.tile import TileContext
    from concourse.masks import make_identity

    f32 = mybir.dt.float32
    bf16 = mybir.dt.bfloat16
    i32 = mybir.dt.int32
    AF = mybir.ActivationFunctionType
    OP = mybir.AluOpType

    nc = Bacc(num_devices=NCORES)
    P_x = nc.declare_dram_parameter("x", [SH, D], bf16, isOutput=False)
    P_Wl = nc.declare_dram_parameter("Wl", [D, D], bf16, isOutput=False)
    P_Wr = nc.declare_dram_parameter("Wr", [D, D], bf16, isOutput=False)
    P_Wrb = nc.declare_dram_parameter("Wrb", [D, 3], f32, isOutput=False)
    P_A = nc.declare_dram_parameter("A", [D, 8], f32, isOutput=False)
    P_blr = nc.declare_dram_parameter("blr", [1, D], f32, isOutput=False)
    P_brr = nc.declare_dram_parameter("brr", [1, D], f32, isOutput=False)
    P_brbr = nc.declare_dram_parameter("brbr", [1, 3], f32, isOutput=False)
    P_es = [nc.declare_dram_parameter(f"es{m}", [W, 128, TW], mybir.dt.uint16,
                                      isOutput=False) for m in (0, 1)]
    P_ct = [nc.declare_dram_parameter(f"ct{m}", [W, 1, 256], f32,
                                      isOutput=False) for m in (0, 1)]
    P_out = nc.declare_dram_parameter("out", [SH, D], bf16, isOutput=True)

    zloc = [nc.dram_tensor(f"zloc{m}", [SH, ZC], f32) for m in (0, 1)]
    zt = [nc.dram_tensor(f"zt{m}", [NPAD, ZC], f32, addr_space="Shared")
          for m in (0, 1)]

    with TileContext(nc) as tc:
        with tc.tile_pool(name="pers", bufs=1) as pers:
            ident = pers.tile([128, 128], f32, tag="ident")
            make_identity(nc, ident[:])
            iota_j = pers.tile([128, TW], i32, tag="iota_j")
            nc.gpsimd.iota(iota_j[:], pattern=[[1, TW]], base=0,
                           channel_multiplier=TW)
            iota_jf = pers.tile([128, TW], f32, tag="iota_jf")
            nc.vector.tensor_copy(iota_jf[:], iota_j[:])
            ones1 = pers.tile([1, 128], f32, tag="ones1")
            nc.vector.memset(ones1[:], 1.0)

            wl_b = pers.tile([128, 128], bf16, tag="wlb")
            nc.sync.dma_start(out=wl_b[:], in_=P_Wl[:, :])
            wl_t = pers.tile([128, 128], f32, tag="wl")
            nc.vector.tensor_copy(wl_t[:], wl_b[:])
            wr_b = pers.tile([128, 128], bf16, tag="wrb16")
            nc.sync.dma_start(out=wr_b[:], in_=P_Wr[:, :])
            wr_t = pers.tile([128, 128], f32, tag="wr")
            nc.vector.tensor_copy(wr_t[:], wr_b[:])
            wrb_t = pers.tile([128, 3], f32, tag="wrb")
            nc.sync.dma_start(out=wrb_t[:], in_=P_Wrb[:, :])
            A_t = pers.tile([128, 8], f32, tag="A")
            nc.sync.dma_start(out=A_t[:], in_=P_A[:, :])
            blr_t = pers.tile([1, 128], f32, tag="blr")
            nc.sync.dma_start(out=blr_t[:], in_=P_blr[:, :])
            brr_t = pers.tile([1, 128], f32, tag="brr")
            nc.sync.dma_start(out=brr_t[:], in_=P_brr[:, :])
            brbr_t = pers.tile([1, 3], f32, tag="brbr")
            nc.sync.dma_start(out=brbr_t[:], in_=P_brbr[:, :])

            r_own = pers.tile([128, W * 128], f32, tag="r_own")
            beta_sb = pers.tile([128, W * 3], f32, tag="beta_sb")
            acc = pers.tile([128, W * 128], f32, tag="acc")

            # ---------------- dense phase (own shard only) ----------------
            with tc.tile_pool(name="dsb", bufs=3) as dsb, \
                 tc.tile_pool(name="dpsA", bufs=2, space="PSUM") as dpsA, \
                 tc.tile_pool(name="dpsB", bufs=1, space="PSUM") as dpsB:
                for g in range(W):
                    sl = slice(g * 128, (g + 1) * 128)
                    xb = dsb.tile([128, 128], bf16, tag="xb")
                    nc.sync.dma_start(out=xb[:], in_=P_x[sl, :])
                    xt = dsb.tile([128, 128], f32, tag="xt")
                    nc.vector.tensor_copy(xt[:], xb[:])
                    xT_ps = dpsB.tile([128, 128], f32, tag="xTp")
                    nc.tensor.transpose(xT_ps[:], xt[:], ident[:])
                    xT = dsb.tile([128, 128], f32, tag="xT")
                    nc.scalar.copy(out=xT[:], in_=xT_ps[:])

                    l_ps = dpsA.tile([128, 128], f32, tag="lp")
                    nc.tensor.matmul(out=l_ps[:], lhsT=xT[:], rhs=wl_t[:],
                                     start=True, stop=False)
                    nc.tensor.matmul(out=l_ps[:], lhsT=ones1[:], rhs=blr_t[:],
                                     start=False, stop=True)

                    lr = dsb.tile([128, 128], f32, tag="lr")
                    nc.vector.tensor_scalar_mul(lr[:], l_ps[:], 0.2)
                    nc.vector.tensor_tensor(out=lr[:], in0=lr[:], in1=l_ps[:],
                                            op=OP.max)
                    lrT_ps = dpsB.tile([128, 128], f32, tag="lrTp")
                    nc.tensor.transpose(lrT_ps[:], lr[:], ident[:])
                    lrT = dsb.tile([128, 128], f32, tag="lrT")
                    nc.scalar.copy(out=lrT[:], in_=lrT_ps[:])
                    ss_ps = dpsB.tile([128, 8], f32, tag="ssp")
                    nc.tensor.matmul(out=ss_ps[:], lhsT=lrT[:], rhs=A_t[:],
                                     start=True, stop=True)
                    u = dsb.tile([128, 8], f32, tag="u")
                    nc.scalar.activation(u[:], ss_ps[:], AF.Exp)

                    zu = dsb.tile([128, 2 * ZC], f32, tag="zu")
                    for m in (0, 1):
                        nc.vector.tensor_tensor(
                            out=zu[:, m * ZC:m * ZC + 128].rearrange(
                                "p (h c) -> p h c", h=4),
                            in0=l_ps[:, :].rearrange("p (h c) -> p h c", h=4),
                            in1=u[:, m * 4:(m + 1) * 4].to_broadcast([128, 4, 32]),
                            op=OP.mult)
                        nc.vector.tensor_copy(zu[:, m * ZC + 128:(m + 1) * ZC],
                                              u[:, m * 4:(m + 1) * 4])
                        nc.sync.dma_start(out=zloc[m][sl, :],
                                          in_=zu[:, m * ZC:(m + 1) * ZC])

                    r_ps = dpsB.tile([128, 128], f32, tag="rp")
                    nc.tensor.matmul(out=r_ps[:], lhsT=xT[:], rhs=wr_t[:],
                                     start=True, stop=False)
                    nc.tensor.matmul(out=r_ps[:], lhsT=ones1[:], rhs=brr_t[:],
                                     start=False, stop=True)
                    nc.scalar.copy(out=r_own[:, sl], in_=r_ps[:])

                    bl_ps = dpsB.tile([128, 3], f32, tag="blp")
                    nc.tensor.matmul(out=bl_ps[:], lhsT=xT[:], rhs=wrb_t[:],
                                     start=True, stop=False)
                    nc.tensor.matmul(out=bl_ps[:], lhsT=ones1[:], rhs=brbr_t[:],
                                     start=False, stop=True)
                    be = dsb.tile([128, 3], f32, tag="be")
                    nc.scalar.activation(be[:], bl_ps[:], AF.Exp)
                    bs = dsb.tile([128, 1], f32, tag="bs")
                    nc.vector.tensor_reduce(out=bs[:], in_=be[:],
                                            axis=mybir.AxisListType.X, op=OP.add)
                    brc = dsb.tile([128, 1], f32, tag="brc")
                    nc.vector.reciprocal(brc[:], bs[:])
                    nc.vector.tensor_tensor(
                        out=beta_sb[:, g * 3:(g + 1) * 3], in0=be[:],
                        in1=brc[:].to_broadcast([128, 3]), op=OP.mult)

            # assemble the full node tables on every core
            for m in (0, 1):
                nc.gpsimd.collective_compute(
                    "AllGather", mybir.AluOpType.bypass,
                    replica_groups=[list(range(NCORES))],
                    ins=[zloc[m][:, :].opt()],
                    outs=[zt[m][:, :].opt()])

            # phase barrier: collapse the dense-phase fan-in (all DMA lanes +
            # engines) into one sync point so edge-phase instructions stay
            # under the ISA per-instruction sync-wait limit
            with tc.tile_critical():
                nc.vector.memset(ones1[:], 1.0)

            # ---------------- edge phase ----------------
            with tc.tile_pool(name="esb", bufs=3) as esb, \
                 tc.tile_pool(name="eps", bufs=2, space="PSUM") as eps, \
                 tc.tile_pool(name="epsB", bufs=2, space="PSUM") as epsB:
                for m in (0, 1):
                    for w in range(W):
                        ws = slice(w * 128, (w + 1) * 128)
                        pes = esb.tile([128, TW], mybir.dt.uint16, tag="pes")
                        nc.sync.dma_start(out=pes[:], in_=P_es[m][w])
                        ct = esb.tile([1, 256], f32, tag="ct")
                        nc.sync.dma_start(out=ct[:], in_=P_ct[m][w])
                        idx = esb.tile([128, TW], i32, tag="idx")
                        nc.vector.tensor_copy(idx[:], pes[:])
                        # broadcast the window's per-lane [start, end) rows
                        # across partitions via ones-outer-product matmuls
                        st_ps = epsB.tile([128, 128], f32, tag="stp")
                        nc.tensor.matmul(out=st_ps[:], lhsT=ones1[:],
                                         rhs=ct[0:1, 0:128], start=True,
                                         stop=True)
                        st_sb = esb.tile([128, 128], f32, tag="stsb")
                        nc.scalar.copy(out=st_sb[:], in_=st_ps[:])
                        en_ps = epsB.tile([128, 128], f32, tag="enp")
                        nc.tensor.matmul(out=en_ps[:], lhsT=ones1[:],
                                         rhs=ct[0:1, 128:256], start=True,
                                         stop=True)
                        en_sb = esb.tile([128, 128], f32, tag="ensb")
                        nc.scalar.copy(out=en_sb[:], in_=en_ps[:])
                        # M[p,t,n] = (start[n] <= j(p,t) < end[n]), j = p*TW+t
                        M = esb.tile([128, TW * 128], f32, tag="M")
                        nc.vector.tensor_tensor(
                            out=M[:].rearrange("p (t n) -> p t n", t=TW),
                            in0=iota_jf[:].to_broadcast([128, TW, 128]),
                            in1=st_sb[:, None, :].to_broadcast([128, TW, 128]),
                            op=OP.is_ge)
                        M2 = esb.tile([128, TW * 128], f32, tag="M2")
                        nc.vector.tensor_tensor(
                            out=M2[:].rearrange("p (t n) -> p t n", t=TW),
                            in0=iota_jf[:].to_broadcast([128, TW, 128]),
                            in1=en_sb[:, None, :].to_broadcast([128, TW, 128]),
                            op=OP.is_ge)
                        nc.vector.tensor_tensor(out=M[:], in0=M[:], in1=M2[:],
                                                op=OP.subtract)
                        gt = esb.tile([128, TW * ZC], f32, tag="gt")
                        for t in range(TW):
                            nc.gpsimd.indirect_dma_start(
                                out=gt[:, t * ZC:(t + 1) * ZC], out_offset=None,
                                in_=zt[m][:, :],
                                in_offset=bass.IndirectOffsetOnAxis(
                                    ap=idx[:, t:t + 1], axis=0))
                        ps = eps.tile([128, ZC], f32, tag="pw")
                        for t in range(TW):
                            nc.tensor.matmul(out=ps[:],
                                             lhsT=M[:, t * 128:(t + 1) * 128],
                                             rhs=gt[:, t * ZC:(t + 1) * ZC],
                                             start=(t == 0), stop=(t == TW - 1))
                        den = esb.tile([128, 4], f32, tag="den")
                        nc.vector.tensor_scalar_add(den[:], ps[:, 128:132], EPS)
                        rec = esb.tile([128, 4], f32, tag="rec")
                        nc.vector.reciprocal(rec[:], den[:])
                        ab = esb.tile([128, 4], f32, tag="ab")
                        nc.vector.tensor_tensor(
                            out=ab[:], in0=rec[:],
                            in1=beta_sb[:, w * 3 + m:w * 3 + m + 1].to_broadcast([128, 4]),
                            op=OP.mult)
                        if m == 0:
                            nc.vector.tensor_tensor(
                                out=acc[:, ws].rearrange("p (h c) -> p h c", h=4),
                                in0=ps[:, 0:128].rearrange("p (h c) -> p h c", h=4),
                                in1=ab[:].to_broadcast([128, 4, 32]), op=OP.mult)
                        else:
                            tmp = esb.tile([128, 128], f32, tag="tmp")
                            nc.vector.tensor_tensor(
                                out=tmp[:].rearrange("p (h c) -> p h c", h=4),
                                in0=ps[:, 0:128].rearrange("p (h c) -> p h c", h=4),
                                in1=ab[:].to_broadcast([128, 4, 32]), op=OP.mult)
                            nc.vector.tensor_tensor(out=acc[:, ws], in0=acc[:, ws],
                                                    in1=tmp[:], op=OP.add)

                for w in range(W):
                    ws = slice(w * 128, (w + 1) * 128)
                    tmp = esb.tile([128, 128], f32, tag="tmp")
                    nc.vector.tensor_tensor(
                        out=tmp[:], in0=r_own[:, ws],
                        in1=beta_sb[:, w * 3 + 2:w * 3 + 3].to_broadcast([128, 128]),
                        op=OP.mult)
                    nc.vector.tensor_tensor(out=tmp[:], in0=tmp[:], in1=acc[:, ws],
                                            op=OP.add)
                    ot = esb.tile([128, 128], bf16, tag="ot")
                    nc.scalar.activation(ot[:], tmp[:], AF.Relu)
                    nc.sync.dma_start(out=P_out[ws, :], in_=ot[:])

    nc.finalize()
    return nc


def _prep_x(inputs):
    """bf16 [NPAD, D]; axis 0 is the per-core concatenation that
    shard_map(in_specs=P("core")) slices back apart."""
    from concourse import mybir
    bf16 = mybir.dt.np(mybir.dt.bfloat16)
    x = np.asarray(inputs["x"], dtype=np.float32)
    x_pad = np.zeros((NPAD, D), dtype=bf16)
    # single-pass f32->bf16 narrowing straight into the padded buffer
    np.copyto(x_pad[:N], x, casting="unsafe")
    return x_pad


def _prep_weights(inputs):
    Wl = np.ascontiguousarray(np.asarray(inputs["Wl"], dtype=np.float32))
    bl = np.asarray(inputs["bl"], dtype=np.float32)
    Wr = np.ascontiguousarray(np.asarray(inputs["Wr"], dtype=np.float32))
    br = np.asarray(inputs["br"], dtype=np.float32)
    Wbeta = np.asarray(inputs["Wbeta"], dtype=np.float32)
    bbeta = np.asarray(inputs["bbeta"], dtype=np.float32)
    attn = np.asarray(inputs["attn"], dtype=np.float32)
    sharpen = np.asarray(inputs["sharpen"], dtype=np.float32)

    Wrb = np.ascontiguousarray(Wr @ Wbeta.T)             # [128, 3]
    brb = (br @ Wbeta.T + bbeta).astype(np.float32)      # [3]
    A = np.zeros((D, 8), dtype=np.float32)
    for m in (0, 1):
        aj = attn[m][:, C:]                              # [H, C]
        for h in range(H):
            A[h * C:(h + 1) * C, m * 4 + h] = aj[h] * sharpen[m]

    from concourse import mybir
    bf16 = mybir.dt.np(mybir.dt.bfloat16)

    def rep(a):
        return np.concatenate([a] * NCORES, axis=0)

    return {
        "Wl": rep(Wl.astype(bf16)), "Wr": rep(Wr.astype(bf16)),
        "Wrb": rep(Wrb), "A": rep(A),
        "blr": rep(bl[None, :]), "brr": rep(br[None, :]),
        "brbr": rep(brb[None, :]),
    }


_EXEC = None


def _get_exec():
    """Build (once) the jitted 8-core executable, mirroring the axon branch
    of bass_utils.run_bass_kernel_spmd / bass2jax.run_bass_via_pjrt."""
    global _EXEC
    if _EXEC is not None:
        return _EXEC
    import jax
    from jax.sharding import Mesh, PartitionSpec
    from jax.experimental.shard_map import shard_map
    from concourse import mybir
    from concourse.bass2jax import (_bass_exec_p, install_neuronx_cc_hook,
                                    partition_id_tensor)

    nc = _build_graph()
    install_neuronx_cc_hook()
    partition_name = (nc.partition_id_tensor.name
                      if nc.partition_id_tensor else None)
    in_names, out_names, out_avals, zero_outs = [], [], [], []
    for alloc in nc.m.functions[0].allocations:
        if not isinstance(alloc, mybir.MemoryLocationSet):
            continue
        name = alloc.memorylocations[0].name
        if alloc.kind == "ExternalInput":
            if name != partition_name:
                in_names.append(name)
        elif alloc.kind == "ExternalOutput":
            out_names.append(name)
            shape = tuple(alloc.tensor_shape)
            dtype = mybir.dt.np(alloc.dtype)
            out_avals.append(jax.core.ShapedArray(shape, dtype))
            zero_outs.append(np.zeros((NCORES * shape[0], *shape[1:]), dtype))
    n_params = len(in_names)
    in_names_all = in_names + out_names + (
        [partition_name] if partition_name else [])
    donate = tuple(range(n_params, n_params + len(out_names)))

    def _body(*args):
        operands = list(args)
        if partition_name is not None:
            operands.append(partition_id_tensor())
        return tuple(_bass_exec_p.bind(
            *operands, out_avals=tuple(out_avals),
            in_names=tuple(in_names_all), out_names=tuple(out_names),
            lowering_input_output_aliases=(), sim_require_finite=True,
            sim_require_nnan=True, nc=nc))

    devices = jax.devices()[:NCORES]
    assert len(devices) == NCORES
    mesh = Mesh(np.asarray(devices), ("core",))
    nspec = n_params + len(out_names)
    jf = jax.jit(
        shard_map(_body, mesh=mesh, in_specs=(PartitionSpec("core"),) * nspec,
                  out_specs=(PartitionSpec("core"),) * len(out_names),
                  check_rep=False),
        donate_argnums=donate, keep_unused=True)
    from jax.sharding import NamedSharding
    _EXEC = {"jf": jf, "in_names": in_names, "out_names": out_names,
             "zero_outs": zero_outs, "donated": None,
             "sharding": NamedSharding(mesh, PartitionSpec("core"))}
    return _EXEC


class _Res:
    exec_time_ns = None


def run(inputs, trace=False):
    import jax
    ex = _get_exec()
    sh = ex["sharding"]
    # upload x (the largest input) asynchronously; the transfer overlaps the
    # host-side edge-table construction below
    g = {"x": jax.device_put(_prep_x(inputs), sh)}
    for k, v in _prep_weights(inputs).items():
        g[k] = jax.device_put(v, sh)
    for m, key in ((0, "edge_index0"), (1, "edge_index1")):
        su, cts = _prep_edges_all(inputs[key])
        g[f"es{m}"] = jax.device_put(su, sh)
        g[f"ct{m}"] = jax.device_put(cts, sh)
    args = [g[name] for name in ex["in_names"]]
    douts = ex["donated"] if ex["donated"] is not None else ex["zero_outs"]
    outs = ex["jf"](*args, *douts)
    # every output element is written by the kernel, so the previous call's
    # output buffers are valid donation targets for the next call
    ex["donated"] = list(outs)
    oi = ex["out_names"].index("out")
    try:
        # queue the host copy behind the running execution
        outs[oi].copy_to_host_async()
    except (AttributeError, RuntimeError):
        pass
    out = np.asarray(outs[oi]).reshape(NPAD, D)[:N]
    return out.astype(np.float32), _Res()


def kernel(**inputs) -> np.ndarray:
    out, _ = run(inputs)
    return out
